# revision 1
# baseline (speedup 1.0000x reference)
"""ComplEx rhs-scoring kernel for Trainium2 (8 NeuronCores).

scores = Re(<lhs * rel, conj(all_ents)>) = q @ ent_emb.T
where q = [q_re, q_im] (complex product of gathered lhs/rel embeddings).

Strategy (tensor-parallel over candidates):
  - host: gather + complex product -> q [B, K] (tiny, exact fp32),
    transpose to qT [K, B]; transpose ent_emb -> eT [K, N]; split eT
    into 8 column slabs [K, N/8] (one per core); replicate qT.
  - device (per core): scores_slab[b, n] = sum_k qT[k, b] * eT[k, n]
    via PE matmuls: lhsT = qT k-tile [128, 128], rhs = eT chunk
    [128, CW], accumulate K/128 = 8 matmuls into PSUM fp32.
  - host: concat slabs along axis 1 -> [B, N] fp32.
"""

import os
import numpy as np

import concourse.bacc as bacc
import concourse.mybir as mybir
import concourse.tile as tile
from concourse.bass_utils import run_bass_kernel_spmd

N_CORES = 8
B = 1024          # batch (queries)
K = 1024          # contraction dim (2 * rank)
N_ENT = 100000    # candidates
NS = N_ENT // N_CORES  # per-core slab width (12500)
P = 128           # partitions
KT = K // P       # k tiles (8)
BT = B // P       # b tiles (8)
CW = 500          # rhs chunk width (one PSUM bank; 25 even chunks per slab)

_DT = {
    "bf16": mybir.dt.bfloat16,
    "f32r": mybir.dt.float32r,
    "f32": mybir.dt.float32,
}


def build_kernel(dt_name, ns=NS, cw=CW, b=B):
    dt_in = _DT[dt_name]
    f32 = mybir.dt.float32
    nc = bacc.Bacc("TRN2", target_bir_lowering=False, debug=False)

    qT = nc.dram_tensor("qT", [K, b], dt_in, kind="ExternalInput")
    eT = nc.dram_tensor("eT", [K, ns], dt_in, kind="ExternalInput")
    out = nc.dram_tensor("out", [b, ns], f32, kind="ExternalOutput")

    bt = b // P
    # chunk widths: full cw chunks plus one remainder chunk
    widths = [cw] * (ns // cw)
    if ns % cw:
        widths.append(ns % cw)
    offs = [sum(widths[:i]) for i in range(len(widths))]
    n_chunks = len(widths)

    # 3D-AP views: put the 128-partition dim first, keep k/b tile index
    # as a middle dim so a whole chunk moves in ONE dma_start (the sync
    # engine's ~0.8us per-issue cost is the scarce resource here).
    eT_r = eT.rearrange("(kt p) n -> p kt n", p=P)    # [128, KT, ns]
    qT_r = qT.rearrange("(kt p) b -> p kt b", p=P)    # [128, KT, b]
    out_r = out.rearrange("(bt p) n -> p bt n", p=P)  # [128, bt, ns]

    with tile.TileContext(nc) as tc:
        with (
            tc.tile_pool(name="qpool", bufs=1) as qpool,
            tc.tile_pool(name="epool", bufs=4) as epool,
            tc.tile_pool(name="pspool", bufs=8, space="PSUM") as pspool,
            tc.tile_pool(name="opool", bufs=2) as opool,
        ):
            # chunk-0 entities first so the first matmuls aren't gated on
            # the full q load. Chunk 0 is split per-k so the transfers fan
            # out across DMA queues (latency matters here; later chunks
            # are single issues since only throughput matters there).
            # q resident in SBUF, loaded in b-quarters: the first quarter
            # unblocks b-tiles 0..1 while the rest streams in. Issue order
            # matches first consumption: et0[k0], q-quarter0, remaining
            # et0 k-slices, remaining q.
            et0 = epool.tile([P, KT * cw], dt_in, tag="et")
            qsb = qpool.tile([P, KT * b], dt_in)
            qsb_r = qsb.rearrange("p (kt b) -> p kt b", kt=KT)
            bq = b // 4

            kh = KT // 2

            def q_quarter(j):
                # two kt-half DMAs per quarter: single-issue DMAs only
                # reach ~160-300GB/s, a pair fans out across queues
                for h in range(2):
                    nc.sync.dma_start(
                        qsb_r[:, h * kh:(h + 1) * kh, j * bq:(j + 1) * bq],
                        qT_r[:, h * kh:(h + 1) * kh, j * bq:(j + 1) * bq],
                    )

            # warm the PE (HAM clock-gate needs ~3.4us of activity) with
            # dummy matmuls on a memset tile while the first DMAs land
            warm = qpool.tile([P, cw], mybir.dt.bfloat16, name="warm")
            nc.gpsimd.memset(warm[:], 0.0)
            ps_w = pspool.tile([P, cw], f32, tag="ps", name="ps_warm")
            for _ in range(10):
                nc.tensor.matmul(ps_w[:], warm[:, 0:P], warm[:],
                                 start=True, stop=True)

            nc.sync.dma_start(et0[:, 0:cw], eT[0:P, 0:cw])
            q_quarter(0)
            for k in range(1, KT):
                nc.sync.dma_start(
                    et0[:, k * cw:(k + 1) * cw],
                    eT[k * P:(k + 1) * P, 0:cw],
                )
            for j in range(1, 4):
                q_quarter(j)

            for c in range(n_chunks):
                w = widths[c]
                off = offs[c]
                if c == 0:
                    et = et0
                else:
                    et = epool.tile([P, KT * w], dt_in, tag="et", name=f"et{c}")
                    et_v = et.rearrange("p (kt w) -> p kt w", kt=KT)
                    for j in range(2):
                        nc.sync.dma_start(
                            et_v[:, j * kh:(j + 1) * kh, :],
                            eT_r[:, j * kh:(j + 1) * kh, off:off + w],
                        )
                ot = opool.tile([P, bt * w], f32, tag="ot", name=f"ot{c}")
                for bi in range(bt):
                    ps = pspool.tile([P, w], f32, tag="ps", name="ps")
                    for k in range(KT):
                        nc.tensor.matmul(
                            ps[:],
                            qsb[:, k * b + bi * P:k * b + (bi + 1) * P],
                            et[:, k * w:(k + 1) * w],
                            start=(k == 0),
                            stop=(k == KT - 1),
                        )
                    if bi % 2 == 0:
                        nc.vector.tensor_copy(ot[:, bi * w:(bi + 1) * w], ps[:])
                    else:
                        nc.scalar.copy(ot[:, bi * w:(bi + 1) * w], ps[:])
                    if bi % 2 == 1:
                        # flush each b-pair as soon as its copies land so
                        # the final chunk's writeback trails the last MM by
                        # only its last pair, not the whole chunk
                        h0 = bi - 1
                        ot_h = ot.rearrange("p (bt w) -> p bt w", bt=bt)
                        # outputs ride the gpsimd DGE queue so writeback
                        # issues never delay entity-chunk prefetch issues
                        # on the sync queue
                        nc.gpsimd.dma_start(
                            out_r[:, h0:bi + 1, off:off + w],
                            ot_h[:, h0:bi + 1, :],
                        )
    nc.compile()
    return nc


def _prep_inputs(x, ent_emb, rel_emb, dt_name):
    x = np.asarray(x)
    ent_emb = np.asarray(ent_emb, dtype=np.float32)
    rel_emb = np.asarray(rel_emb, dtype=np.float32)
    r = ent_emb.shape[1] // 2
    lhs = ent_emb[x[:, 0]]
    rel = rel_emb[x[:, 1]]
    lre, lim = lhs[:, :r], lhs[:, r:]
    rre, rim = rel[:, :r], rel[:, r:]
    q = np.empty((x.shape[0], 2 * r), np.float32)
    q[:, :r] = lre * rre - lim * rim
    q[:, r:] = lre * rim + lim * rre

    if dt_name == "bf16":
        import ml_dtypes
        np_dt = ml_dtypes.bfloat16
    else:
        np_dt = np.float32

    qT = np.ascontiguousarray(q.T).astype(np_dt)           # [K, B]
    eT = np.ascontiguousarray(ent_emb.T).astype(np_dt)     # [K, N]
    in_maps = [
        {"qT": qT, "eT": np.ascontiguousarray(eT[:, i * NS:(i + 1) * NS])}
        for i in range(N_CORES)
    ]
    return in_maps


def run(x, ent_emb, rel_emb, dt_name=None, trace=False, **spmd_kwargs):
    dt_name = dt_name or os.environ.get("KERNEL_DT", "f32r")
    nc = build_kernel(dt_name)
    in_maps = _prep_inputs(x, ent_emb, rel_emb, dt_name)
    res = run_bass_kernel_spmd(
        nc, in_maps, list(range(N_CORES)), trace=trace, **spmd_kwargs
    )
    outs = [res.results[i]["out"] for i in range(N_CORES)]
    return np.concatenate(outs, axis=1), res


def kernel(x, ent_emb, rel_emb):
    out, _ = run(x, ent_emb, rel_emb)
    return out

